# revision 15
# baseline (speedup 1.0000x reference)
"""Trainium2 Bass kernel for batched windowed multi-head attention.

Shapes: x (8, 64, 256, 512) f32, H=8 heads, D=64.
Sharding: data-parallel over batch dim B=8 -> 1 batch row per NeuronCore.

v8 design (v3 attention core + residual-fp8 DoubleRow projections):
- q/k/v projections run as fp8e4m3 DoubleRow matmuls with full residual
  compensation: x = x8 + xr and 16W = W8 + Wr (all fp8), computed as
  x8@W8 + xr@W8 + x8@Wr.  Each DR instruction contracts two 128-row
  K-groups at 0.5 cycles/row, so the 3-term sum costs 0.75x the bf16
  single-pass while keeping ~bf16 accuracy (dropped xr@Wr ~ 0.03%).
- Attention core is v3's: bf16 scores (heads row-packed), exp on ACT,
  exp(mask+pos_bias) precomputed on host ("emp") with the softmax as
  p = exp(scores) * emp, attn@v + ones-matmul denominators in bf16,
  reciprocal+normalize on DVE, bf16 out-projection.
- GPSIMD cannot touch PSUM, so all PSUM evacuations ride ACT/DVE
  (scalar.activation with per-partition bias + scale slots absorb the
  1/16 weight-scale compensation for free).
"""
import os
import numpy as np
import ml_dtypes

import concourse.bass as bass
import concourse.mybir as mybir
import concourse.tile as tile
from concourse import bacc
from concourse.bass_utils import run_bass_kernel_spmd

B, W, S, E = 8, 64, 256, 512
H, D = 8, 64
SCALE = D ** -0.5
NCORES = 8
F32 = mybir.dt.float32
BF16 = mybir.dt.bfloat16
F8 = mybir.dt.float8e4
NPBF16 = ml_dtypes.bfloat16
NPF8 = ml_dtypes.float8_e4m3
AOp = mybir.AluOpType
AF = mybir.ActivationFunctionType
DR = mybir.MatmulPerfMode.DoubleRow

WS = 16.0  # host-side weight scale for fp8 range; undone in the evacs


def _emit(nc, tc, ctx, n_g, d):
    """Emit the per-core program: n_g groups of 2 windows of MHA."""
    const = ctx.enter_context(tc.tile_pool(name="const", bufs=1))

    # --- one-time constants ---
    w_sb = {}
    for name in ("wq8", "wqr", "wk8", "wkr", "wv8", "wvr"):
        t = const.tile([128, 4, E], F8, tag=name)
        nc.sync.dma_start(t[:], d[name][:])
        w_sb[name] = t
    wp = const.tile([128, 4, E], BF16, tag="wp")
    nc.sync.dma_start(wp[:], d["wp"][:])
    bq_col = const.tile([128, 4], F32)
    nc.sync.dma_start(bq_col[:], d["bq"][:])
    bk_col = const.tile([128, 4], F32)
    nc.sync.dma_start(bk_col[:], d["bk"][:])
    bp_bc = const.tile([128, 4, 2, S], F32)
    nc.sync.dma_start(bp_bc[:], d["bp"][:])
    ones_den = const.tile([128, 64], BF16)
    nc.gpsimd.memset(ones_den[:], 1.0)

    # --- pools ---
    xt_p = ctx.enter_context(tc.tile_pool(name="xt", bufs=3))
    emp_p = ctx.enter_context(tc.tile_pool(name="emp", bufs=3))
    qkv_p = ctx.enter_context(tc.tile_pool(name="qkv", bufs=2))
    pe_p = ctx.enter_context(tc.tile_pool(name="pe", bufs=6))
    pp_p = ctx.enter_context(tc.tile_pool(name="pp", bufs=6))
    rec_p = ctx.enter_context(tc.tile_pool(name="rec", bufs=6))
    zt_p = ctx.enter_context(tc.tile_pool(name="zt", bufs=2))
    outs_p = ctx.enter_context(tc.tile_pool(name="outs", bufs=3))

    ps_proj = ctx.enter_context(tc.tile_pool(name="ps_proj", bufs=2, space="PSUM"))
    ps_sc = ctx.enter_context(tc.tile_pool(name="ps_sc", bufs=2, space="PSUM"))
    ps_zd = ctx.enter_context(tc.tile_pool(name="ps_zd", bufs=2, space="PSUM"))

    def phase_a(g):
        """DMA + projections for window pair g; returns tiles + chunk closures."""
        x2 = xt_p.tile([128, 2, 4, 2, S], F8, tag="x2", name=f"x2{g}")
        nc.sync.dma_start(x2[:], d["x"][g])
        emp_t = [None, None]
        for wi in range(2):
            emp_t[wi] = emp_p.tile([128, H, 2, S], BF16, tag=f"emp{wi}",
                                   name=f"emp{g}_{wi}")
            nc.sync.dma_start(emp_t[wi][:], d["emp"][2 * g + wi])

        qT = qkv_p.tile([128, 4, 2, S], BF16, tag="qT", name=f"qT{g}")
        kT = qkv_p.tile([128, 4, 2, S], BF16, tag="kT", name=f"kT{g}")
        vA = [qkv_p.tile([128, 2, H, D], BF16, tag=f"vA{wi}", name=f"vA{g}_{wi}")
              for wi in range(2)]

        # 3-term residual DR: x8@W8 + xr@W8 + x8@Wr  (x2[:,0]=x8, x2[:,1]=xr)
        def qk_chunk(wt, dstT, bias_col, scl, oc):
            pp = ps_proj.tile([128, 2, S], F32, tag="pj", name=f"pp{g}_{wt}_{oc}")
            for wi in range(2):
                terms = [(0, w_sb[wt + "8"]), (1, w_sb[wt + "8"]),
                         (0, w_sb[wt + "r"])]
                n = 0
                for xi, wm in terms:
                    for g2 in range(2):
                        nc.tensor.matmul(pp[:, wi],
                                         wm[:, 2 * g2:2 * g2 + 2,
                                            oc * 128:(oc + 1) * 128],
                                         x2[:, xi, 2 * g2:2 * g2 + 2, wi],
                                         start=(n == 0), stop=(n == 5),
                                         perf_mode=DR)
                        n += 1
            nc.scalar.activation(dstT[:, oc], pp[:], AF.Identity,
                                 bias=bias_col[:, oc:oc + 1], scale=scl)

        def v_chunk(wi, sc):
            pv = ps_proj.tile([128, E], F32, tag="pj", name=f"pv{g}_{wi}_{sc}")
            for eh in range(2):
                terms = [(0, w_sb["wv8"]), (1, w_sb["wv8"]), (0, w_sb["wvr"])]
                n = 0
                for xi, wm in terms:
                    for g2 in range(2):
                        nc.tensor.matmul(pv[:, eh * 256:(eh + 1) * 256],
                                         x2[:, xi, 2 * g2:2 * g2 + 2, wi,
                                            sc * 128:(sc + 1) * 128],
                                         wm[:, 2 * g2:2 * g2 + 2,
                                            eh * 256:(eh + 1) * 256],
                                         start=(n == 0), stop=(n == 5),
                                         perf_mode=DR)
                        n += 1
            nc.scalar.mul(vA[wi][:, sc],
                          pv[:].rearrange("p (h v) -> p h v", h=H), 1.0 / WS)

        chunks = []
        for oc in range(4):
            chunks.append(lambda oc=oc: qk_chunk("wq", qT, bq_col, SCALE / WS, oc))
            chunks.append(lambda oc=oc: qk_chunk("wk", kT, bk_col, 1.0 / WS, oc))
        for wi in range(2):
            for sc in range(2):
                chunks.append(lambda wi=wi, sc=sc: v_chunk(wi, sc))
        return (qT, kT, vA, emp_t), chunks

    def phase_b(g, qT, kT, vA, emp_t):
        """Attention closures + output-projection tail for window pair g."""
        zT = zt_p.tile([128, 4, 2, S], BF16, tag="zT", name=f"zT{g}")
        pair_state = {}

        def pair_front(wi, k):
            # transposed scores, heads 2k / 2k+1 row-packed
            scp = ps_sc.tile([128, 2, 2, S], F32, tag="sc", name=f"sc{g}_{wi}_{k}")
            for jc in range(2):
                for a in range(2):
                    prow = a * 64
                    nc.tensor.matmul(scp[:, a, jc],
                                     kT[prow:prow + 64, k, wi, jc * 128:(jc + 1) * 128],
                                     qT[prow:prow + 64, k, wi], start=True, stop=True)
            pexp = pe_p.tile([128, 2, 2, S], BF16, tag="pexp", name=f"pe{g}_{wi}_{k}")
            nc.scalar.activation(pexp[:], scp[:], AF.Exp)
            p_sb = pp_p.tile([128, 2, 2, S], BF16, tag="p", name=f"p{g}_{wi}_{k}")
            eng = nc.gpsimd if (wi * 4 + k) % 2 == 0 else nc.vector
            eng.tensor_tensor(p_sb[:], pexp[:], emp_t[wi][:, 2 * k:2 * k + 2],
                              AOp.mult)
            pair_state[(wi, k)] = p_sb

        def pair_back(wi, k):
            p_sb = pair_state.pop((wi, k))
            # za (half 0) + pre-broadcast denominators (half 1); each
            # accumulation group runs to completion before the next group's
            # start=True (it clears has_written for the whole bank); groups on
            # alternating col-halves still overlap in the PE.
            zd = ps_zd.tile([128, 2, S], F32, tag="zd", name=f"zd{g}_{wi}_{k}")
            for a in range(2):
                for jc in range(2):
                    nc.tensor.matmul(zd[a * 64:(a + 1) * 64, 0],
                                     vA[wi][:, jc, 2 * k + a], p_sb[:, a, jc],
                                     start=(jc == 0), stop=(jc == 1))
            for a in range(2):
                for jc in range(2):
                    nc.tensor.matmul(zd[a * 64:(a + 1) * 64, 1],
                                     ones_den[:], p_sb[:, a, jc],
                                     start=(jc == 0), stop=(jc == 1))
            rec = rec_p.tile([128, S], F32, tag="rec", name=f"rec{g}_{wi}_{k}")
            nc.vector.reciprocal_approx_fast(rec[:], zd[:, 1])
            nc.vector.tensor_tensor(zT[:, k, wi], zd[:, 0], rec[:], AOp.mult)

        def tail():
            outs = outs_p.tile([128, 4, 2, S], BF16, tag="osb", name=f"osb{g}")
            for oc in range(4):
                po = ps_proj.tile([128, 2, S], F32, tag="pj", name=f"po{g}_{oc}")
                for ec in range(4):
                    nc.tensor.matmul(po[:], wp[:, ec, oc * 128:(oc + 1) * 128],
                                     zT[:, ec], start=(ec == 0), stop=(ec == 3))
                nc.vector.scalar_tensor_tensor(
                    outs[:, oc], po[:], 0.0, bp_bc[:, oc], AOp.bypass, AOp.add)
            nc.sync.dma_start(d["out"][g], outs[:])

        fronts = [lambda wi=wi, k=k: pair_front(wi, k)
                  for wi in range(2) for k in range(4)]
        backs = [lambda wi=wi, k=k: pair_back(wi, k)
                 for wi in range(2) for k in range(4)]
        return fronts, backs, tail

    prev = None
    for g in range(n_g):
        cur, chunks = phase_a(g)
        if prev is not None:
            fronts, backs, tail = phase_b(g - 1, *prev)
            # Interleave: score matmuls early (feed exp/emp-mult pipeline);
            # projection chunks of group g fill the vector-engine latency,
            # spread between the attn@v stages to cover the p-tile chain.
            seq = []
            ci = 0
            for i in range(4):
                seq.extend([fronts[2 * i], fronts[2 * i + 1]])
                seq.extend(chunks[ci:ci + 2]); ci += 2
            for i in range(4):
                seq.append(backs[i])
                seq.append(chunks[ci]); ci += 1
            seq.extend(backs[4:])
            seq.extend(chunks[ci:])
            seq.append(tail)
            for fn in seq:
                fn()
        else:
            for fn in chunks:
                fn()
        prev = cur
    fronts, backs, tail = phase_b(n_g - 1, *prev)
    for fn in fronts:
        fn()
    for fn in backs:
        fn()
    tail()


def _build(n_g):
    nc = bacc.Bacc("TRN2", target_bir_lowering=False, debug=False)
    d = {
        "x": nc.dram_tensor("x", [n_g, 128, 2, 4, 2, S], F8, kind="ExternalInput"),
        "emp": nc.dram_tensor("emp", [2 * n_g, 128, H, 2, S], BF16,
                              kind="ExternalInput"),
        "wq8": nc.dram_tensor("wq8", [128, 4, E], F8, kind="ExternalInput"),
        "wqr": nc.dram_tensor("wqr", [128, 4, E], F8, kind="ExternalInput"),
        "wk8": nc.dram_tensor("wk8", [128, 4, E], F8, kind="ExternalInput"),
        "wkr": nc.dram_tensor("wkr", [128, 4, E], F8, kind="ExternalInput"),
        "wv8": nc.dram_tensor("wv8", [128, 4, E], F8, kind="ExternalInput"),
        "wvr": nc.dram_tensor("wvr", [128, 4, E], F8, kind="ExternalInput"),
        "wp": nc.dram_tensor("wp", [128, 4, E], BF16, kind="ExternalInput"),
        "bq": nc.dram_tensor("bq", [128, 4], F32, kind="ExternalInput"),
        "bk": nc.dram_tensor("bk", [128, 4], F32, kind="ExternalInput"),
        "bp": nc.dram_tensor("bp", [128, 4, 2, S], F32, kind="ExternalInput"),
        "out": nc.dram_tensor("out", [n_g, 128, 4, 2, S], BF16,
                              kind="ExternalOutput"),
    }
    from contextlib import ExitStack
    with tile.TileContext(nc) as tc, ExitStack() as ctx:
        _emit(nc, tc, ctx, n_g, d)
    nc.compile()
    return nc


_NC_CACHE = {}


def _get_nc(n_g):
    if n_g not in _NC_CACHE:
        _NC_CACHE[n_g] = _build(n_g)
    return _NC_CACHE[n_g]


def _host_prep(mask, Wq, bq, Wk, bk, Wv, bv, Wp, bp, pos_bias, n_w):
    """Shared (replicated) tensors, host-side layout prep."""
    f = np.float32

    def wlay_split(wmat):
        # [out,in] torch Linear weight -> 16*W.T as fp8 hi + fp8 residual,
        # each [128(in%128), ic, o]
        wt = np.asarray(wmat, f).T * WS
        w8 = wt.astype(NPF8)
        wr = (wt - w8.astype(f)).astype(NPF8)
        lay = lambda a: np.ascontiguousarray(
            a.astype(f).reshape(4, 128, E).transpose(1, 0, 2)).astype(NPF8)
        return lay(w8), lay(wr)

    def wlay16(wmat):
        wt = np.asarray(wmat, f).T
        return np.ascontiguousarray(
            wt.reshape(4, 128, E).transpose(1, 0, 2)).astype(NPBF16)

    def bcol(bvec, scale=1.0):
        # [o] -> [128(o%128), oc] f32
        return np.ascontiguousarray(
            (np.asarray(bvec, f) * scale).reshape(4, 128).T)

    # v bias folded into the output bias: out += bv @ Wp.T  (softmax rows
    # sum to 1), so v needs no bias on-device.
    bp_eff = np.asarray(bp, f) + np.asarray(Wp, f) @ np.asarray(bv, f)
    bp_b = np.ascontiguousarray(np.broadcast_to(
        bcol(bp_eff)[:, :, None, None], (128, 4, 2, S)).astype(f))

    # emp = exp(mask^T + pos_bias^T), [w, 128(j%128), h, jc, i] bf16
    mT = np.asarray(mask, f)[0, :n_w, 0].transpose(0, 2, 1)       # [w, j, i]
    pT = np.asarray(pos_bias, f).transpose(0, 2, 1)               # [h, j, i]
    emp = np.exp(mT[:, None] + pT[None])                          # [w, h, j, i]
    emp = emp.reshape(n_w, H, 2, 128, S).transpose(0, 3, 1, 2, 4)
    emp = np.ascontiguousarray(emp).astype(NPBF16)

    wq8, wqr = wlay_split(Wq)
    wk8, wkr = wlay_split(Wk)
    wv8, wvr = wlay_split(Wv)
    return {
        "wq8": wq8, "wqr": wqr, "wk8": wk8, "wkr": wkr,
        "wv8": wv8, "wvr": wvr, "wp": wlay16(Wp),
        "bq": bcol(bq, SCALE), "bk": bcol(bk), "bp": bp_b,
        "emp": emp,
    }


def _x_lay(xc, n_w):
    # x[core] [w, s, e] -> [g, 128(e%128), 2(hi/lo), ic, wi, s] fp8 pair
    xt = np.asarray(xc, np.float32)[:n_w].transpose(0, 2, 1)      # [w, e, s]
    xt = xt.reshape(n_w // 2, 2, 4, 128, S).transpose(0, 3, 2, 1, 4)
    x8 = xt.astype(NPF8)
    xr = (xt - x8.astype(np.float32)).astype(NPF8)
    return np.ascontiguousarray(
        np.stack([x8, xr], axis=2))                               # [g,128,2,4,2,S]


def kernel(x, mask, Wq, bq, Wk, bk, Wv, bv, Wp, bp, pos_bias, _trace=False):
    n_w = int(os.environ.get("KERNEL_NW", W))
    assert n_w % 2 == 0, "window count must be even (processed in pairs)"
    n_cores = NCORES
    x = np.asarray(x, np.float32)
    shared = _host_prep(mask, Wq, bq, Wk, bk, Wv, bv, Wp, bp, pos_bias, n_w)

    in_maps = []
    for c in range(n_cores):
        m = dict(shared)
        m["x"] = _x_lay(x[c % B], n_w)
        in_maps.append(m)

    nc = _get_nc(n_w // 2)
    res = run_bass_kernel_spmd(nc, in_maps, list(range(n_cores)), trace=_trace,
                               tmpdir=(os.environ.get("KERNEL_TRACE_DIR") if _trace else None))
    # out [g, 128(o%128), oc, wi, s] bf16 -> [w, s, o] f32
    outs = []
    for c in range(B):
        o = np.asarray(res.results[c]["out"]).astype(np.float32)
        o = o.transpose(0, 3, 2, 1, 4).reshape(n_w, E, S).transpose(0, 2, 1)
        outs.append(np.ascontiguousarray(o))
    out = np.stack(outs, axis=0)
    if _trace:
        kernel._last_exec_time_ns = res.exec_time_ns
        kernel._last_results = res
    return out


# revision 32
# speedup vs baseline: 1.1926x; 1.1926x over previous
"""Trainium2 Bass kernel for batched windowed multi-head attention.

Shapes: x (8, 64, 256, 512) f32, H=8 heads, D=64.
Sharding: data-parallel over batch dim B=8 -> 1 batch row per NeuronCore.

v3 design:
- x transposed on the HOST -> xT arrives via DMA (no PE transposes).
- exp(mask + pos_bias) precomputed on the HOST per (window, head), DMA'd
  bf16 ("emp"); softmax p = exp(scores) * emp.
- All matmuls bf16 (fp32 PSUM accumulation).
- Windows processed in PAIRS: weight-stationary projections (q/k/out)
  stream both windows' activations as one N=512 moving operand, halving
  matmul + evacuation op counts.
- Scores transposed (j on partitions), heads 2k/2k+1 row-packed (d=64
  contraction -> PE rows 0-63 / 64-127 run concurrently).
- attn@v col-packed per head pair; a ones[128,64] stationary produces
  PRE-BROADCAST softmax denominators in the same PSUM bank.
- v bias folded into the output-projection bias on the host
  (bp_eff = bp + Wp @ bv; softmax rows sum to 1).
- q/k evacuations ride on ScalarE (per-partition bias); out evac on
  VectorE scalar_tensor_tensor; emp-multiplies split GpSimd/VectorE.
"""
import os
import numpy as np
import ml_dtypes

import concourse.bass as bass
import concourse.mybir as mybir
import concourse.tile as tile
from concourse import bacc
from concourse.bass_utils import run_bass_kernel_spmd

B, W, S, E = 8, 64, 256, 512
H, D = 8, 64
SCALE = D ** -0.5
NCORES = 8
F32 = mybir.dt.float32
BF16 = mybir.dt.bfloat16
F8 = mybir.dt.float8e4
NPBF16 = ml_dtypes.bfloat16
NPF8 = ml_dtypes.float8_e4m3
AOp = mybir.AluOpType
AF = mybir.ActivationFunctionType
DR = mybir.MatmulPerfMode.DoubleRow


def _emit(nc, tc, ctx, n_g, d):
    """Emit the per-core program: n_g groups of 2 windows of MHA."""
    const = ctx.enter_context(tc.tile_pool(name="const", bufs=1))

    # --- one-time constants ---
    # per-oc weight DMAs so the first projection chunk only waits on its
    # own slice instead of all 2 MB of weights
    w_sb = {}
    for name in ("wq", "wk", "wv", "wp"):
        t = const.tile([128, 4, E], BF16, tag=name)
        for oc in range(4):
            nc.sync.dma_start(t[:, :, oc * 128:(oc + 1) * 128],
                              d[name][:, :, oc * 128:(oc + 1) * 128])
        w_sb[name] = t
    bq_col = const.tile([128, 4], F32)
    nc.sync.dma_start(bq_col[:], d["bq"][:])
    bk_col = const.tile([128, 4], F32)
    nc.sync.dma_start(bk_col[:], d["bk"][:])
    bp_bc = const.tile([128, 4, 2, S], F32)
    nc.sync.dma_start(bp_bc[:], d["bp"][:])
    ones_den = const.tile([128, 64], BF16)
    nc.gpsimd.memset(ones_den[:], 1.0)

    # --- pools ---
    xt_p = ctx.enter_context(tc.tile_pool(name="xt", bufs=3))
    emp_p = ctx.enter_context(tc.tile_pool(name="emp", bufs=3))
    qkv_p = ctx.enter_context(tc.tile_pool(name="qkv", bufs=2))
    pe_p = ctx.enter_context(tc.tile_pool(name="pe", bufs=6))
    pp_p = ctx.enter_context(tc.tile_pool(name="pp", bufs=6))
    rec_p = ctx.enter_context(tc.tile_pool(name="rec", bufs=6))
    zt_p = ctx.enter_context(tc.tile_pool(name="zt", bufs=2))
    outs_p = ctx.enter_context(tc.tile_pool(name="outs", bufs=3))

    ps_proj = ctx.enter_context(tc.tile_pool(name="ps_proj", bufs=2, space="PSUM"))
    ps_sc = ctx.enter_context(tc.tile_pool(name="ps_sc", bufs=2, space="PSUM"))
    ps_zd = ctx.enter_context(tc.tile_pool(name="ps_zd", bufs=2, space="PSUM"))

    def phase_a(g):
        """DMA + projections for window pair g; returns tiles + chunk closures."""
        xT = xt_p.tile([128, 4, 2, S], BF16, tag="xT", name=f"xT{g}")
        nc.sync.dma_start(xT[:], d["x"][g])
        emp_t = [None, None]
        for wi in range(2):
            emp_t[wi] = emp_p.tile([128, H, 2, S], BF16, tag=f"emp{wi}",
                                   name=f"emp{g}_{wi}")
            nc.sync.dma_start(emp_t[wi][:], d["emp"][2 * g + wi])

        qT = qkv_p.tile([128, 4, 2, S], BF16, tag="qT", name=f"qT{g}")
        kT = qkv_p.tile([128, 4, 2, S], BF16, tag="kT", name=f"kT{g}")
        vA = [qkv_p.tile([128, 2, H, D], BF16, tag=f"vA{wi}", name=f"vA{g}_{wi}")
              for wi in range(2)]

        def qk_chunk(wt, dstT, bias_col, oc):
            # both windows ride in one N=512 moving operand
            pp = ps_proj.tile([128, 2, S], F32, tag="pj", name=f"pp{g}_{wt}_{oc}")
            for ic in range(4):
                nc.tensor.matmul(pp[:], w_sb[wt][:, ic, oc * 128:(oc + 1) * 128],
                                 xT[:, ic], start=(ic == 0), stop=(ic == 3))
            nc.scalar.activation(dstT[:, oc], pp[:], AF.Identity,
                                 bias=bias_col[:, oc:oc + 1])

        def v_chunk(wi, sc):
            pv = ps_proj.tile([128, E], F32, tag="pj", name=f"pv{g}_{wi}_{sc}")
            for ic in range(4):
                nc.tensor.matmul(pv[:], xT[:, ic, wi, sc * 128:(sc + 1) * 128],
                                 w_sb["wv"][:, ic], start=(ic == 0), stop=(ic == 3))
            nc.scalar.copy(vA[wi][:, sc], pv[:].rearrange("p (h v) -> p h v", h=H))

        chunks = []
        for oc in range(4):
            chunks.append(lambda oc=oc: qk_chunk("wq", qT, bq_col, oc))
            chunks.append(lambda oc=oc: qk_chunk("wk", kT, bk_col, oc))
        for wi in range(2):
            for sc in range(2):
                chunks.append(lambda wi=wi, sc=sc: v_chunk(wi, sc))
        return (qT, kT, vA, emp_t), chunks

    def phase_b(g, qT, kT, vA, emp_t):
        """Attention closures + output-projection tail for window pair g."""
        zT = zt_p.tile([128, 4, 2, S], BF16, tag="zT", name=f"zT{g}")
        pair_state = {}

        def pair_front(wi, k):
            # transposed scores, heads 2k / 2k+1 row-packed
            scp = ps_sc.tile([128, 2, 2, S], F32, tag="sc", name=f"sc{g}_{wi}_{k}")
            for jc in range(2):
                for a in range(2):
                    prow = a * 64
                    nc.tensor.matmul(scp[:, a, jc],
                                     kT[prow:prow + 64, k, wi, jc * 128:(jc + 1) * 128],
                                     qT[prow:prow + 64, k, wi], start=True, stop=True)
            pexp = pe_p.tile([128, 2, 2, S], BF16, tag="pexp", name=f"pe{g}_{wi}_{k}")
            nc.scalar.activation(pexp[:], scp[:], AF.Exp)
            p_sb = pp_p.tile([128, 2, 2, S], BF16, tag="p", name=f"p{g}_{wi}_{k}")
            eng = nc.gpsimd if (wi * 4 + k) % 2 == 0 else nc.vector
            eng.tensor_tensor(p_sb[:], pexp[:], emp_t[wi][:, 2 * k:2 * k + 2],
                              AOp.mult)
            pair_state[(wi, k)] = p_sb

        def pair_back(wi, k):
            p_sb = pair_state.pop((wi, k))
            # za (half 0) + pre-broadcast denominators (half 1); each
            # accumulation group runs to completion before the next group's
            # start=True (it clears has_written for the whole bank); groups on
            # alternating col-halves still overlap in the PE.
            zd = ps_zd.tile([128, 2, S], F32, tag="zd", name=f"zd{g}_{wi}_{k}")
            for a in range(2):
                for jc in range(2):
                    nc.tensor.matmul(zd[a * 64:(a + 1) * 64, 0],
                                     vA[wi][:, jc, 2 * k + a], p_sb[:, a, jc],
                                     start=(jc == 0), stop=(jc == 1))
            for a in range(2):
                for jc in range(2):
                    nc.tensor.matmul(zd[a * 64:(a + 1) * 64, 1],
                                     ones_den[:], p_sb[:, a, jc],
                                     start=(jc == 0), stop=(jc == 1))
            rec = rec_p.tile([128, S], F32, tag="rec", name=f"rec{g}_{wi}_{k}")
            nc.vector.reciprocal_approx_fast(rec[:], zd[:, 1])
            nc.vector.tensor_tensor(zT[:, k, wi], zd[:, 0], rec[:], AOp.mult)

        # out-projection streamed by contraction chunk: po(oc=0,1) accumulate
        # ec=k as soon as both windows' z for head-pair k lands (woven between
        # the last backs), so the tail never waits on the whole zT.
        tst = {}

        def tail_step(ec):
            if ec == 0:
                tst["outs"] = outs_p.tile([128, 4, 2, S], BF16, tag="osb",
                                          name=f"osb{g}")
                for oc in range(2):
                    tst[oc] = ps_proj.tile([128, 2, S], F32, tag="pj",
                                           name=f"po{g}_{oc}")
            for oc in range(2):
                nc.tensor.matmul(tst[oc][:],
                                 w_sb["wp"][:, ec, oc * 128:(oc + 1) * 128],
                                 zT[:, ec], start=(ec == 0), stop=(ec == 3))
            if ec == 3:
                for oc in range(2):
                    nc.vector.scalar_tensor_tensor(
                        tst["outs"][:, oc], tst[oc][:], 0.0, bp_bc[:, oc],
                        AOp.bypass, AOp.add)

        def tail_rest():
            outs = tst.pop("outs")
            for oc in range(2, 4):
                po = ps_proj.tile([128, 2, S], F32, tag="pj", name=f"po{g}_{oc}")
                for ec in range(4):
                    nc.tensor.matmul(po[:], w_sb["wp"][:, ec, oc * 128:(oc + 1) * 128],
                                     zT[:, ec], start=(ec == 0), stop=(ec == 3))
                nc.vector.scalar_tensor_tensor(
                    outs[:, oc], po[:], 0.0, bp_bc[:, oc], AOp.bypass, AOp.add)
            nc.sync.dma_start(d["out"][g], outs[:])

        fronts = [lambda wi=wi, k=k: pair_front(wi, k)
                  for wi in range(2) for k in range(4)]
        backs = [lambda wi=wi, k=k: pair_back(wi, k)
                 for wi in range(2) for k in range(4)]
        tails = [lambda ec=ec: tail_step(ec) for ec in range(4)] + [tail_rest]
        return fronts, backs, tails

    def weave(fronts, backs, tails, chunks):
        # score matmuls early (feed exp/emp-mult pipeline); projection chunks
        # of the NEXT group fill engine latency between stages; out-proj ec
        # chunks stream between the wi=1 backs.
        seq = []
        ci = 0
        for i in range(4):
            seq.extend([fronts[2 * i], fronts[2 * i + 1]])
            seq.extend(chunks[ci:ci + 2]); ci += 2
        for i in range(4):
            seq.append(backs[i])
            seq.extend(chunks[ci:ci + 1]); ci += 1
        for i in range(4):
            seq.append(backs[4 + i])
            seq.append(tails[i])
        seq.extend(chunks[ci:])
        seq.append(tails[4])
        return seq

    prev = None
    for g in range(n_g):
        cur, chunks = phase_a(g)
        if prev is not None:
            for fn in weave(*phase_b(g - 1, *prev), chunks):
                fn()
        else:
            for fn in chunks:
                fn()
        prev = cur
    for fn in weave(*phase_b(n_g - 1, *prev), []):
        fn()


def _build(n_g):
    nc = bacc.Bacc("TRN2", target_bir_lowering=False, debug=False)
    d = {
        "x": nc.dram_tensor("x", [n_g, 128, 4, 2, S], BF16, kind="ExternalInput"),
        "emp": nc.dram_tensor("emp", [2 * n_g, 128, H, 2, S], BF16,
                              kind="ExternalInput"),
        "wq": nc.dram_tensor("wq", [128, 4, E], BF16, kind="ExternalInput"),
        "wk": nc.dram_tensor("wk", [128, 4, E], BF16, kind="ExternalInput"),
        "wv": nc.dram_tensor("wv", [128, 4, E], BF16, kind="ExternalInput"),
        "wp": nc.dram_tensor("wp", [128, 4, E], BF16, kind="ExternalInput"),
        "bq": nc.dram_tensor("bq", [128, 4], F32, kind="ExternalInput"),
        "bk": nc.dram_tensor("bk", [128, 4], F32, kind="ExternalInput"),
        "bp": nc.dram_tensor("bp", [128, 4, 2, S], F32, kind="ExternalInput"),
        "out": nc.dram_tensor("out", [n_g, 128, 4, 2, S], BF16,
                              kind="ExternalOutput"),
    }
    from contextlib import ExitStack
    with tile.TileContext(nc) as tc, ExitStack() as ctx:
        _emit(nc, tc, ctx, n_g, d)
    nc.compile()
    return nc


_NC_CACHE = {}


def _get_nc(n_g):
    if n_g not in _NC_CACHE:
        _NC_CACHE[n_g] = _build(n_g)
    return _NC_CACHE[n_g]


def _host_prep(mask, Wq, bq, Wk, bk, Wv, bv, Wp, bp, pos_bias, n_w):
    """Shared (replicated) tensors, host-side layout prep."""
    f = np.float32

    def wlay(wmat, scale=1.0):
        npdt = NPBF16
        # [out,in] torch Linear weight -> [128(e%128), ic, o], e=ic*128+p
        wt = np.asarray(wmat, f).T * scale
        return np.ascontiguousarray(
            wt.reshape(4, 128, E).transpose(1, 0, 2)).astype(npdt)

    def bcol(bvec, scale=1.0):
        # [o] -> [128(o%128), oc] f32
        return np.ascontiguousarray(
            (np.asarray(bvec, f) * scale).reshape(4, 128).T)

    # v bias folded into the output bias: out += bv @ Wp.T  (softmax rows
    # sum to 1), so v needs no bias on-device.
    bp_eff = np.asarray(bp, f) + np.asarray(Wp, f) @ np.asarray(bv, f)
    bp_b = np.ascontiguousarray(np.broadcast_to(
        bcol(bp_eff)[:, :, None, None], (128, 4, 2, S)).astype(f))

    # emp = exp(mask^T + pos_bias^T), [w, 128(j%128), h, jc, i] bf16
    mT = np.asarray(mask, f)[0, :n_w, 0].transpose(0, 2, 1)       # [w, j, i]
    pT = np.asarray(pos_bias, f).transpose(0, 2, 1)               # [h, j, i]
    emp = np.exp(mT[:, None] + pT[None])                          # [w, h, j, i]
    emp = emp.reshape(n_w, H, 2, 128, S).transpose(0, 3, 1, 2, 4)
    emp = np.ascontiguousarray(emp).astype(NPBF16)

    return {
        "wq": wlay(Wq, scale=SCALE), "wk": wlay(Wk), "wv": wlay(Wv),
        "wp": wlay(Wp),
        "bq": bcol(bq, SCALE), "bk": bcol(bk), "bp": bp_b,
        "emp": emp,
    }


def _x_lay(xc, n_w):
    # x[core] [w, s, e] -> [g, 128(e%128), ic, wi, s] fp8e4, e=ic*128+p
    xt = np.asarray(xc, np.float32)[:n_w].transpose(0, 2, 1)      # [w, e, s]
    xt = xt.reshape(n_w // 2, 2, 4, 128, S).transpose(0, 3, 2, 1, 4)
    return np.ascontiguousarray(xt).astype(NPBF16)


def kernel(x, mask, Wq, bq, Wk, bk, Wv, bv, Wp, bp, pos_bias, _trace=False):
    n_w = int(os.environ.get("KERNEL_NW", W))
    assert n_w % 2 == 0, "window count must be even (processed in pairs)"
    n_cores = NCORES
    x = np.asarray(x, np.float32)
    shared = _host_prep(mask, Wq, bq, Wk, bk, Wv, bv, Wp, bp, pos_bias, n_w)

    in_maps = []
    for c in range(n_cores):
        m = dict(shared)
        m["x"] = _x_lay(x[c % B], n_w)
        in_maps.append(m)

    nc = _get_nc(n_w // 2)
    res = run_bass_kernel_spmd(nc, in_maps, list(range(n_cores)), trace=_trace,
                               tmpdir=(os.environ.get("KERNEL_TRACE_DIR") if _trace else None))
    # out [g, 128(o%128), oc, wi, s] bf16 -> [w, s, o] f32
    outs = []
    for c in range(B):
        o = np.asarray(res.results[c]["out"]).astype(np.float32)
        o = o.transpose(0, 3, 2, 1, 4).reshape(n_w, E, S).transpose(0, 2, 1)
        outs.append(np.ascontiguousarray(o))
    out = np.stack(outs, axis=0)
    if _trace:
        kernel._last_exec_time_ns = res.exec_time_ns
        kernel._last_results = res
    return out



# revision 38
# speedup vs baseline: 1.2520x; 1.0498x over previous
"""Trainium2 Bass kernel for batched windowed multi-head attention.

Shapes: x (8, 64, 256, 512) f32, H=8 heads, D=64.
Sharding: data-parallel over batch dim B=8 -> 1 batch row per NeuronCore.

v3 design:
- x transposed on the HOST -> xT arrives via DMA (no PE transposes).
- exp(mask + pos_bias) precomputed on the HOST per (window, head), DMA'd
  bf16 ("emp"); softmax p = exp(scores) * emp.
- All matmuls bf16 (fp32 PSUM accumulation).
- Windows processed in PAIRS: weight-stationary projections (q/k/out)
  stream both windows' activations as one N=512 moving operand, halving
  matmul + evacuation op counts.
- Scores transposed (j on partitions), heads 2k/2k+1 row-packed (d=64
  contraction -> PE rows 0-63 / 64-127 run concurrently).
- attn@v col-packed per head pair; a ones[128,64] stationary produces
  PRE-BROADCAST softmax denominators in the same PSUM bank.
- v bias folded into the output-projection bias on the host
  (bp_eff = bp + Wp @ bv; softmax rows sum to 1).
- q/k evacuations ride on VectorE tensor_scalar (per-partition bias) so
  the ScalarE queue stays clear for exp (which gates the score-PSUM
  ping-pong); out evac on ScalarE (per-partition bias); v evac ScalarE;
  emp-multiplies split GpSimd/VectorE.
"""
import os
import numpy as np
import ml_dtypes

import concourse.bass as bass
import concourse.mybir as mybir
import concourse.tile as tile
from concourse import bacc
from concourse.bass_utils import run_bass_kernel_spmd

B, W, S, E = 8, 64, 256, 512
H, D = 8, 64
SCALE = D ** -0.5
NCORES = 8
F32 = mybir.dt.float32
BF16 = mybir.dt.bfloat16
F8 = mybir.dt.float8e4
NPBF16 = ml_dtypes.bfloat16
NPF8 = ml_dtypes.float8_e4m3
AOp = mybir.AluOpType
AF = mybir.ActivationFunctionType
DR = mybir.MatmulPerfMode.DoubleRow


def _emit(nc, tc, ctx, n_g, d):
    """Emit the per-core program: n_g groups of 2 windows of MHA."""
    const = ctx.enter_context(tc.tile_pool(name="const", bufs=1))

    # --- one-time constants ---
    w_sb = {}
    for name in ("wq", "wk", "wv", "wp"):
        t = const.tile([128, 4, E], BF16, tag=name)
        nc.sync.dma_start(t[:], d[name][:])
        w_sb[name] = t
    bq_col = const.tile([128, 4], F32)
    nc.sync.dma_start(bq_col[:], d["bq"][:])
    bk_col = const.tile([128, 4], F32)
    nc.sync.dma_start(bk_col[:], d["bk"][:])
    bp_col = const.tile([128, 4], F32)
    nc.sync.dma_start(bp_col[:], d["bp"][:])
    ones_den = const.tile([128, 64], BF16)
    nc.gpsimd.memset(ones_den[:], 1.0)

    # --- pools ---
    xt_p = ctx.enter_context(tc.tile_pool(name="xt", bufs=3))
    emp_p = ctx.enter_context(tc.tile_pool(name="emp", bufs=3))
    qkv_p = ctx.enter_context(tc.tile_pool(name="qkv", bufs=2))
    pe_p = ctx.enter_context(tc.tile_pool(name="pe", bufs=6))
    pp_p = ctx.enter_context(tc.tile_pool(name="pp", bufs=6))
    rec_p = ctx.enter_context(tc.tile_pool(name="rec", bufs=6))
    zt_p = ctx.enter_context(tc.tile_pool(name="zt", bufs=2))
    outs_p = ctx.enter_context(tc.tile_pool(name="outs", bufs=3))

    ps_proj = ctx.enter_context(tc.tile_pool(name="ps_proj", bufs=2, space="PSUM"))
    ps_sc = ctx.enter_context(tc.tile_pool(name="ps_sc", bufs=2, space="PSUM"))
    ps_zd = ctx.enter_context(tc.tile_pool(name="ps_zd", bufs=2, space="PSUM"))

    def phase_a(g):
        """DMA + projections for window pair g; returns tiles + chunk closures."""
        xT = xt_p.tile([128, 4, 2, S], BF16, tag="xT", name=f"xT{g}")
        nc.sync.dma_start(xT[:], d["x"][g])
        emp_t = [None, None]
        for wi in range(2):
            emp_t[wi] = emp_p.tile([128, H, 2, S], BF16, tag=f"emp{wi}",
                                   name=f"emp{g}_{wi}")
            nc.sync.dma_start(emp_t[wi][:], d["emp"][2 * g + wi])

        qT = qkv_p.tile([128, 4, 2, S], BF16, tag="qT", name=f"qT{g}")
        kT = qkv_p.tile([128, 4, 2, S], BF16, tag="kT", name=f"kT{g}")
        vA = [qkv_p.tile([128, 2, H, D], BF16, tag=f"vA{wi}", name=f"vA{g}_{wi}")
              for wi in range(2)]

        def qk_chunk(wt, dstT, bias_col, oc):
            # both windows ride in one N=512 moving operand
            pp = ps_proj.tile([128, 2, S], F32, tag="pj", name=f"pp{g}_{wt}_{oc}")
            for ic in range(4):
                nc.tensor.matmul(pp[:], w_sb[wt][:, ic, oc * 128:(oc + 1) * 128],
                                 xT[:, ic], start=(ic == 0), stop=(ic == 3))
            nc.vector.tensor_scalar(dstT[:, oc], pp[:],
                                    bias_col[:, oc:oc + 1], None, AOp.add)

        def v_chunk(wi, sc):
            pv = ps_proj.tile([128, E], F32, tag="pj", name=f"pv{g}_{wi}_{sc}")
            for ic in range(4):
                nc.tensor.matmul(pv[:], xT[:, ic, wi, sc * 128:(sc + 1) * 128],
                                 w_sb["wv"][:, ic], start=(ic == 0), stop=(ic == 3))
            nc.scalar.copy(vA[wi][:, sc], pv[:].rearrange("p (h v) -> p h v", h=H))

        chunks = []
        for oc in range(4):
            chunks.append(lambda oc=oc: qk_chunk("wq", qT, bq_col, oc))
            chunks.append(lambda oc=oc: qk_chunk("wk", kT, bk_col, oc))
        for wi in range(2):
            for sc in range(2):
                chunks.append(lambda wi=wi, sc=sc: v_chunk(wi, sc))
        return (qT, kT, vA, emp_t), chunks

    def phase_b(g, qT, kT, vA, emp_t):
        """Attention closures + output-projection tail for window pair g."""
        zT = zt_p.tile([128, 4, 2, S], BF16, tag="zT", name=f"zT{g}")
        pair_state = {}

        def pair_front(wi, k):
            # transposed scores, heads 2k / 2k+1 row-packed
            scp = ps_sc.tile([128, 2, 2, S], F32, tag="sc", name=f"sc{g}_{wi}_{k}")
            for jc in range(2):
                for a in range(2):
                    prow = a * 64
                    nc.tensor.matmul(scp[:, a, jc],
                                     kT[prow:prow + 64, k, wi, jc * 128:(jc + 1) * 128],
                                     qT[prow:prow + 64, k, wi], start=True, stop=True)
            pexp = pe_p.tile([128, 2, 2, S], BF16, tag="pexp", name=f"pe{g}_{wi}_{k}")
            nc.scalar.activation(pexp[:], scp[:], AF.Exp)
            p_sb = pp_p.tile([128, 2, 2, S], BF16, tag="p", name=f"p{g}_{wi}_{k}")
            eng = nc.gpsimd if (wi * 4 + k) % 2 == 0 else nc.vector
            eng.tensor_tensor(p_sb[:], pexp[:], emp_t[wi][:, 2 * k:2 * k + 2],
                              AOp.mult)
            pair_state[(wi, k)] = p_sb

        def pair_back(wi, k):
            p_sb = pair_state.pop((wi, k))
            # za (half 0) + pre-broadcast denominators (half 1); each
            # accumulation group runs to completion before the next group's
            # start=True (it clears has_written for the whole bank); groups on
            # alternating col-halves still overlap in the PE.
            zd = ps_zd.tile([128, 2, S], F32, tag="zd", name=f"zd{g}_{wi}_{k}")
            for a in range(2):
                for jc in range(2):
                    nc.tensor.matmul(zd[a * 64:(a + 1) * 64, 0],
                                     vA[wi][:, jc, 2 * k + a], p_sb[:, a, jc],
                                     start=(jc == 0), stop=(jc == 1))
            for a in range(2):
                for jc in range(2):
                    nc.tensor.matmul(zd[a * 64:(a + 1) * 64, 1],
                                     ones_den[:], p_sb[:, a, jc],
                                     start=(jc == 0), stop=(jc == 1))
            rec = rec_p.tile([128, S], F32, tag="rec", name=f"rec{g}_{wi}_{k}")
            nc.vector.reciprocal_approx_fast(rec[:], zd[:, 1])
            nc.vector.tensor_tensor(zT[:, k, wi], zd[:, 0], rec[:], AOp.mult)

        def tail():
            outs = outs_p.tile([128, 4, 2, S], BF16, tag="osb", name=f"osb{g}")
            for oc in range(4):
                po = ps_proj.tile([128, 2, S], F32, tag="pj", name=f"po{g}_{oc}")
                for ec in range(4):
                    nc.tensor.matmul(po[:], w_sb["wp"][:, ec, oc * 128:(oc + 1) * 128],
                                     zT[:, ec], start=(ec == 0), stop=(ec == 3))
                nc.scalar.activation(outs[:, oc], po[:], AF.Identity,
                                     bias=bp_col[:, oc:oc + 1])
            nc.sync.dma_start(d["out"][g], outs[:])

        fronts = [lambda wi=wi, k=k: pair_front(wi, k)
                  for wi in range(2) for k in range(4)]
        backs = [lambda wi=wi, k=k: pair_back(wi, k)
                 for wi in range(2) for k in range(4)]
        return fronts, backs, tail

    prev = None
    for g in range(n_g):
        cur, chunks = phase_a(g)
        if prev is not None:
            fronts, backs, tail = phase_b(g - 1, *prev)
            # Interleave: score matmuls early (feed exp/emp-mult pipeline);
            # projection chunks of group g fill the vector-engine latency,
            # spread between the attn@v stages to cover the p-tile chain.
            seq = []
            ci = 0
            for i in range(4):
                seq.extend([fronts[2 * i], fronts[2 * i + 1]])
                seq.extend(chunks[ci:ci + 2]); ci += 2
            for i in range(4):
                seq.append(backs[i])
                seq.append(chunks[ci]); ci += 1
            seq.extend(backs[4:])
            seq.extend(chunks[ci:])
            seq.append(tail)
            for fn in seq:
                fn()
        else:
            for fn in chunks:
                fn()
        prev = cur
    fronts, backs, tail = phase_b(n_g - 1, *prev)
    for fn in fronts:
        fn()
    for fn in backs:
        fn()
    tail()


def _build(n_g):
    nc = bacc.Bacc("TRN2", target_bir_lowering=False, debug=False)
    d = {
        "x": nc.dram_tensor("x", [n_g, 128, 4, 2, S], BF16, kind="ExternalInput"),
        "emp": nc.dram_tensor("emp", [2 * n_g, 128, H, 2, S], BF16,
                              kind="ExternalInput"),
        "wq": nc.dram_tensor("wq", [128, 4, E], BF16, kind="ExternalInput"),
        "wk": nc.dram_tensor("wk", [128, 4, E], BF16, kind="ExternalInput"),
        "wv": nc.dram_tensor("wv", [128, 4, E], BF16, kind="ExternalInput"),
        "wp": nc.dram_tensor("wp", [128, 4, E], BF16, kind="ExternalInput"),
        "bq": nc.dram_tensor("bq", [128, 4], F32, kind="ExternalInput"),
        "bk": nc.dram_tensor("bk", [128, 4], F32, kind="ExternalInput"),
        "bp": nc.dram_tensor("bp", [128, 4], F32, kind="ExternalInput"),
        "out": nc.dram_tensor("out", [n_g, 128, 4, 2, S], BF16,
                              kind="ExternalOutput"),
    }
    from contextlib import ExitStack
    with tile.TileContext(nc) as tc, ExitStack() as ctx:
        _emit(nc, tc, ctx, n_g, d)
    nc.compile()
    return nc


_NC_CACHE = {}


def _get_nc(n_g):
    if n_g not in _NC_CACHE:
        _NC_CACHE[n_g] = _build(n_g)
    return _NC_CACHE[n_g]


def _host_prep(mask, Wq, bq, Wk, bk, Wv, bv, Wp, bp, pos_bias, n_w):
    """Shared (replicated) tensors, host-side layout prep."""
    f = np.float32

    def wlay(wmat, scale=1.0):
        npdt = NPBF16
        # [out,in] torch Linear weight -> [128(e%128), ic, o], e=ic*128+p
        wt = np.asarray(wmat, f).T * scale
        return np.ascontiguousarray(
            wt.reshape(4, 128, E).transpose(1, 0, 2)).astype(npdt)

    def bcol(bvec, scale=1.0):
        # [o] -> [128(o%128), oc] f32
        return np.ascontiguousarray(
            (np.asarray(bvec, f) * scale).reshape(4, 128).T)

    # v bias folded into the output bias: out += bv @ Wp.T  (softmax rows
    # sum to 1), so v needs no bias on-device.
    bp_eff = np.asarray(bp, f) + np.asarray(Wp, f) @ np.asarray(bv, f)

    # emp = exp(mask^T + pos_bias^T), [w, 128(j%128), h, jc, i] bf16
    mT = np.asarray(mask, f)[0, :n_w, 0].transpose(0, 2, 1)       # [w, j, i]
    pT = np.asarray(pos_bias, f).transpose(0, 2, 1)               # [h, j, i]
    emp = np.exp(mT[:, None] + pT[None])                          # [w, h, j, i]
    emp = emp.reshape(n_w, H, 2, 128, S).transpose(0, 3, 1, 2, 4)
    emp = np.ascontiguousarray(emp).astype(NPBF16)

    return {
        "wq": wlay(Wq, scale=SCALE), "wk": wlay(Wk), "wv": wlay(Wv),
        "wp": wlay(Wp),
        "bq": bcol(bq, SCALE), "bk": bcol(bk), "bp": bcol(bp_eff),
        "emp": emp,
    }


def _x_lay(xc, n_w):
    # x[core] [w, s, e] -> [g, 128(e%128), ic, wi, s] fp8e4, e=ic*128+p
    xt = np.asarray(xc, np.float32)[:n_w].transpose(0, 2, 1)      # [w, e, s]
    xt = xt.reshape(n_w // 2, 2, 4, 128, S).transpose(0, 3, 2, 1, 4)
    return np.ascontiguousarray(xt).astype(NPBF16)


def kernel(x, mask, Wq, bq, Wk, bk, Wv, bv, Wp, bp, pos_bias, _trace=False):
    n_w = int(os.environ.get("KERNEL_NW", W))
    assert n_w % 2 == 0, "window count must be even (processed in pairs)"
    n_cores = NCORES
    x = np.asarray(x, np.float32)
    shared = _host_prep(mask, Wq, bq, Wk, bk, Wv, bv, Wp, bp, pos_bias, n_w)

    in_maps = []
    for c in range(n_cores):
        m = dict(shared)
        m["x"] = _x_lay(x[c % B], n_w)
        in_maps.append(m)

    nc = _get_nc(n_w // 2)
    res = run_bass_kernel_spmd(nc, in_maps, list(range(n_cores)), trace=_trace,
                               tmpdir=(os.environ.get("KERNEL_TRACE_DIR") if _trace else None))
    # out [g, 128(o%128), oc, wi, s] bf16 -> [w, s, o] f32
    outs = []
    for c in range(B):
        o = np.asarray(res.results[c]["out"]).astype(np.float32)
        o = o.transpose(0, 3, 2, 1, 4).reshape(n_w, E, S).transpose(0, 2, 1)
        outs.append(np.ascontiguousarray(o))
    out = np.stack(outs, axis=0)
    if _trace:
        kernel._last_exec_time_ns = res.exec_time_ns
        kernel._last_results = res
    return out

